# revision 23
# baseline (speedup 1.0000x reference)
"""Trainium2 Bass kernel for variable-window left/right max pooling.

out[b, c, t] = max(feat[b, c, max(t-L,0) : t+1]) + max(feat[b, c, t : min(t+R,T)])
with L = max(0, round(reg[b,t,0])), R = clip(round(reg[b,t,1]), 1, T).

Strategy (2 batches/core, pure data parallel over 8 cores, fp16 on device):
  - sparse table built in c-major layout [c%128, lev, cb, t] on the DVE:
    one full-width tensor_tensor(max) per level against a flat shifted view
    (reads that run past a level slab land in never-queried entries, so no
    pads or memsets are needed).
  - each level is transposed to t-major [t%128, tt, c] on the otherwise-idle
    PE (16 identity-matmul transposes per level into fp16 PSUM), copied
    PSUM->SBUF (levels 1-3 on ACT, 4-5 on DVE), and stored to a per-batch
    DRAM row table [NLEV*T, C]. Level 0 rows come from a host-transposed
    featT input via a DRAM->DRAM copy. Batch 0's stores issue on the sync
    queue, batch 1's on the ACT queue so neither batch stalls the other.
  - the 4 RMQ terms per token (left a/b, right a/b) are fetched with 16
    hardware indirect-DMA row gathers (offsets [128, 1] int32, one 2KB row
    per partition), landing t-major in SBUF, gated on a DRAM readback
    barrier after the level stores.
  - combine on DVE: max(la, lb) + max(ra, rb); store out t-major; host
    transposes [B, T, C] -> [B, C, T].
"""

import sys
import types

import numpy as np


def _install_profile_shim():
    if "antenv.axon_hooks" in sys.modules:
        return
    try:
        hooks = types.ModuleType("antenv.axon_hooks")
        hooks._hook = None
        hooks.set_axon_ntff_profile_hook = lambda h: setattr(hooks, "_hook", h)
        hooks.get_axon_ntff_profile_hook = lambda: hooks._hook
        sys.modules["antenv.axon_hooks"] = hooks
        import antenv

        antenv.axon_hooks = hooks
        from trn_agent_boot.trn_boot import _ntff_profile_via_ctypes

        hooks.set_axon_ntff_profile_hook(
            _ntff_profile_via_ctypes("/opt/axon/libaxon_pjrt.so")
        )
    except Exception:
        pass


_install_profile_shim()

import concourse.bacc as bacc
import concourse.bass as bass
import concourse.mybir as mybir
from concourse.bass_utils import run_bass_kernel_spmd

B, C, T = 16, 1024, 256
N_CORES = 8
BPC = B // N_CORES
NLEV = 6   # sparse-table levels 0..5 (windows up to 33)
NG = 8     # indirect gathers per batch: 4 terms x 2 token chunks
CB = C // 128
LW = CB * T  # free elems per level slab per partition (2048)

_LOG2 = np.zeros(65, dtype=np.int64)
for _n in range(1, 65):
    _LOG2[_n] = _n.bit_length() - 1

_CACHE = {}
LAST_RESULT = None


def _build_graph():
    if "nc" in _CACHE:
        return _CACHE["nc"]

    nc = bacc.Bacc("TRN2", target_bir_lowering=False, debug=False,
                   num_devices=N_CORES)
    f16 = mybir.dt.float16
    i16 = mybir.dt.int16
    i32 = mybir.dt.int32

    feat_ext = nc.dram_tensor("feat16", [BPC, C, T], f16,
                              kind="ExternalInput").ap()
    featT_ext = nc.dram_tensor("featT", [BPC, T, C], f16,
                               kind="ExternalInput").ap()
    offs_ext = nc.dram_tensor("offs", [128, BPC * NG], i32,
                              kind="ExternalInput").ap()
    out_ext = nc.dram_tensor("out", [BPC, T, C], f16,
                             kind="ExternalOutput").ap()

    tbl = [nc.dram_tensor(f"tbl{b}", [NLEV * T, C], f16).ap()
           for b in range(BPC)]

    # c-major table: flat free layout [lev][cb][t]
    cbuf = [nc.alloc_sbuf_tensor(f"cbuf{b}", [128, NLEV * LW], f16).ap()
            for b in range(BPC)]
    # t-major staging, one slot per level 1..5 (slot k-1)
    tbuf = [nc.alloc_sbuf_tensor(f"tbuf{b}", [128, NLEV - 1, 2, C], f16).ap()
            for b in range(BPC)]
    gout = [nc.alloc_sbuf_tensor(f"gout{b}", [128, 4, 2, C], f16).ap()
            for b in range(BPC)]
    obuf = [nc.alloc_sbuf_tensor(f"obuf{b}", [128, 2, C], f16).ap()
            for b in range(BPC)]
    offs_sb = nc.alloc_sbuf_tensor("offs_sb", [128, BPC * NG], i32).ap()
    ident = nc.alloc_sbuf_tensor("ident", [128, 128], f16).ap()
    rb_sb = [nc.alloc_sbuf_tensor(f"rb_sb{b}", [NLEV, 64], f16).ap()
             for b in range(BPC)]

    pbuf = [[nc.alloc_psum_tensor(f"pbuf{b}_{j}", [128, 2, C], f16).ap()
             for j in range(2)] for b in range(BPC)]

    with nc.Block() as block:
        s_inc = [nc.alloc_semaphore(f"s_inc{b}") for b in range(BPC)]
        s_ino = nc.alloc_semaphore("s_ino")
        s_id = nc.alloc_semaphore("s_id")
        s_bld = [nc.alloc_semaphore(f"s_bld{b}") for b in range(BPC)]
        s_pe = [nc.alloc_semaphore(f"s_pe{b}") for b in range(BPC)]
        s_cpa = [nc.alloc_semaphore(f"s_cpa{b}") for b in range(BPC)]
        s_cpv = [nc.alloc_semaphore(f"s_cpv{b}") for b in range(BPC)]
        s_st = [nc.alloc_semaphore(f"s_st{b}") for b in range(BPC)]
        s_stq = [nc.alloc_semaphore(f"s_stq{b}") for b in range(BPC)]
        s_rb = [nc.alloc_semaphore(f"s_rb{b}") for b in range(BPC)]
        s_g = [nc.alloc_semaphore(f"s_g{b}") for b in range(BPC)]
        s_cmb = [nc.alloc_semaphore(f"s_cmb{b}") for b in range(BPC)]
        s_out = [nc.alloc_semaphore(f"s_out{b}") for b in range(BPC)]

        def emit_store(eng, b, k):
            if k <= 3:
                eng.wait_ge(s_cpa[b], k)
            else:
                eng.wait_ge(s_cpv[b], k - 3)
            eng.dma_start(
                out=tbl[b][k * T:(k + 1) * T].rearrange(
                    "(tt p) c -> p tt c", p=128),
                in_=tbuf[b][:, k - 1, :, :],
            ).then_inc(s_stq[b], 16)

        def emit_readback(eng, b):
            eng.wait_ge(s_st[b], 16)
            eng.wait_ge(s_stq[b], 80)
            # DRAM readback barrier: touch one chunk of every level's rows
            # before the gathers read the table
            eng.dma_start(
                out=rb_sb[b],
                in_=tbl[b].rearrange("(l t) c -> l t c", t=T)[:, 0, 0:64],
            ).then_inc(s_rb[b], 16)

        @block.sync
        def _(sync):
            for b in range(BPC):
                sync.dma_start(
                    out=cbuf[b][:, 0:LW].rearrange(
                        "p (cb t) -> p cb t", cb=CB),
                    in_=feat_ext[b].rearrange("(cb p) t -> p cb t", p=128),
                ).then_inc(s_inc[b], 16)
            sync.dma_start(out=offs_sb, in_=offs_ext).then_inc(s_ino, 16)
            for b in range(BPC):
                # level-0 rows straight from host featT (DRAM -> DRAM)
                sync.dma_start(out=tbl[b][0:T, :],
                               in_=featT_ext[b]).then_inc(s_st[b], 16)
            for k in range(1, NLEV):
                emit_store(sync, 0, k)
            emit_readback(sync, 0)
            for b in range(BPC):
                sync.wait_ge(s_cmb[b], 3)
                sync.dma_start(
                    out=out_ext[b].rearrange("(tt p) c -> p tt c", p=128),
                    in_=obuf[b][:, :, :],
                ).then_inc(s_out[b], 16)
            for b in range(BPC):
                sync.wait_ge(s_out[b], 16)

        @block.vector
        def _(vector):
            def build(b, k):
                s = 1 << (k - 1)
                if k == 1:
                    vector.wait_ge(s_inc[b], 16)
                o = (k - 1) * LW
                vector.tensor_tensor(
                    out=cbuf[b][:, k * LW:(k + 1) * LW],
                    in0=cbuf[b][:, o:o + LW],
                    in1=cbuf[b][:, o + s:o + s + LW],
                    op=mybir.AluOpType.max,
                ).then_inc(s_bld[b], 1)

            def copy45(b, k):
                # PSUM->SBUF copy for levels 4/5 on the DVE
                vector.wait_ge(s_pe[b], 16 * k)
                vector.tensor_copy(
                    tbuf[b][:, k - 1, :, :], pbuf[b][k % 2][:, :, :],
                ).then_inc(s_cpv[b], 1)

            for k in range(1, NLEV):
                build(0, k)
            build(1, 1)
            build(1, 2)
            copy45(0, 4)
            build(1, 3)
            copy45(0, 5)
            build(1, 4)
            build(1, 5)
            copy45(1, 4)
            copy45(1, 5)
            for b in range(BPC):
                # all gathers of this batch (order-insensitive count)
                vector.wait_ge(s_g[b], 128)
                vector.tensor_tensor(
                    out=gout[b][:, 0, :, :],
                    in0=gout[b][:, 0, :, :],
                    in1=gout[b][:, 1, :, :],
                    op=mybir.AluOpType.max,
                ).then_inc(s_cmb[b], 1)
                vector.tensor_tensor(
                    out=gout[b][:, 2, :, :],
                    in0=gout[b][:, 2, :, :],
                    in1=gout[b][:, 3, :, :],
                    op=mybir.AluOpType.max,
                ).then_inc(s_cmb[b], 1)
                vector.tensor_tensor(
                    out=obuf[b][:, :, :],
                    in0=gout[b][:, 0, :, :],
                    in1=gout[b][:, 2, :, :],
                    op=mybir.AluOpType.add,
                ).then_inc(s_cmb[b], 1)

        @block.tensor
        def _(tensor):
            tensor.wait_ge(s_id, 2)
            for b in range(BPC):
                for k in range(1, NLEV):
                    tensor.wait_ge(s_bld[b], k)
                    if k >= 3:
                        tensor.wait_ge(s_cpa[b], k - 2)
                    for tt in range(2):
                        for cb in range(CB):
                            off = k * LW + cb * T + tt * 128
                            tensor.transpose(
                                out=pbuf[b][k % 2][:, tt, cb * 128:
                                                   (cb + 1) * 128],
                                in_=cbuf[b][:, off:off + 128],
                                identity=ident,
                            ).then_inc(s_pe[b], 1)

        @block.scalar
        def _(scalar):
            def copy_act(b, k):
                scalar.wait_ge(s_pe[b], 16 * k)
                scalar.copy(
                    out=tbuf[b][:, k - 1, :, :],
                    in_=pbuf[b][k % 2][:, :, :],
                ).then_inc(s_cpa[b], 1)

            for k in range(1, 4):
                copy_act(0, k)
            copy_act(1, 1)
            emit_store(scalar, 1, 1)
            copy_act(1, 2)
            emit_store(scalar, 1, 2)
            copy_act(1, 3)
            emit_store(scalar, 1, 3)
            emit_store(scalar, 1, 4)
            emit_store(scalar, 1, 5)
            emit_readback(scalar, 1)

        @block.gpsimd
        def _(gpsimd):
            gpsimd.memset(ident, 0.0).then_inc(s_id, 1)
            gpsimd.affine_select(
                out=ident,
                in_=ident,
                compare_op=mybir.AluOpType.not_equal,
                fill=1.0,
                base=0,
                pattern=[[-1, 128]],
                channel_multiplier=1,
            ).then_inc(s_id, 1)
            gpsimd.wait_ge(s_ino, 16)
            for b in range(BPC):
                gpsimd.wait_ge(s_rb[b], 16)
                for g in range(NG):
                    gpsimd.indirect_dma_start(
                        out=gout[b][:, g // 2, g % 2, :],
                        out_offset=None,
                        in_=tbl[b],
                        in_offset=bass.IndirectOffsetOnAxis(
                            ap=offs_sb[:, b * NG + g:b * NG + g + 1], axis=0),
                    ).then_inc(s_g[b], 16)

    nc.compile()
    _CACHE["nc"] = nc
    return nc


def _host_rows(reg):
    """Table row indices [B, 4, T] for terms (la, lb, ra, rb);
    row(level, x) = level * T + x."""
    t = np.arange(T, dtype=np.int64)[None, :]

    rl = np.maximum(np.round(reg[:, :, 0]).astype(np.int64), 0)
    l_left = np.maximum(t - rl, 0)
    len_l = t + 1 - l_left
    k_l = np.where(len_l <= 64, _LOG2[np.minimum(len_l, 64)],
                   np.floor(np.log2(len_l)).astype(np.int64))
    p_l = (1 << k_l).astype(np.int64)
    la = k_l * T + l_left
    lb = k_l * T + (t + 1 - p_l)

    rr = np.clip(np.round(reg[:, :, 1]).astype(np.int64), 1, T)
    r_right = np.minimum(t + rr, T)
    len_r = r_right - t
    k_r = np.where(len_r <= 64, _LOG2[np.minimum(len_r, 64)],
                   np.floor(np.log2(len_r)).astype(np.int64))
    p_r = (1 << k_r).astype(np.int64)
    ra = k_r * T + (t + np.zeros_like(rr))
    rb = k_r * T + (r_right - p_r)

    rows = np.stack([la, lb, ra, rb], axis=1)  # [B, 4, T]
    assert rows.min() >= 0 and rows.max() < NLEV * T, (rows.min(), rows.max())
    return rows


def _wrap_idxs(flat):
    n = flat.shape[0]
    blk = flat.reshape(n // 16, 16).T
    return np.tile(blk, (8, 1))


def kernel(feat: np.ndarray, reg: np.ndarray) -> np.ndarray:
    global LAST_RESULT
    feat = np.ascontiguousarray(feat, dtype=np.float32)
    reg = np.ascontiguousarray(reg, dtype=np.float32)
    assert feat.shape == (B, C, T) and reg.shape == (B, T, 2)

    feat16 = feat.astype(np.float16)
    featT = np.ascontiguousarray(feat16.transpose(0, 2, 1))  # [B, T, C]
    rows = _host_rows(reg)  # [B, 4, T]
    # indirect offsets: offs[b][p, g], g = term*2 + tt, token t = tt*128 + p
    offs = rows.reshape(B, 4, 2, 128).reshape(B, 8, 128)
    offs = np.ascontiguousarray(offs.transpose(0, 2, 1)).astype(np.int32)

    nc = _build_graph()
    in_maps = []
    for i in range(N_CORES):
        sl = slice(i * BPC, (i + 1) * BPC)
        in_maps.append({
            "feat16": np.ascontiguousarray(feat16[sl]),
            "featT": np.ascontiguousarray(featT[sl]),
            "offs": np.ascontiguousarray(
                offs[sl].transpose(1, 0, 2).reshape(128, BPC * NG)),
        })

    res = run_bass_kernel_spmd(nc, in_maps, list(range(N_CORES)))
    LAST_RESULT = res
    outT = np.concatenate([res.results[i]["out"] for i in range(N_CORES)],
                          axis=0)  # [B, T, C] fp16
    return np.ascontiguousarray(outT.transpose(0, 2, 1)).astype(np.float32)


# revision 24
# speedup vs baseline: 1.0374x; 1.0374x over previous
"""Trainium2 Bass kernel for variable-window left/right max pooling.

out[b, c, t] = max(feat[b, c, max(t-L,0) : t+1]) + max(feat[b, c, t : min(t+R,T)])
with L = max(0, round(reg[b,t,0])), R = clip(round(reg[b,t,1]), 1, T).

Strategy (2 batches/core, pure data parallel over 8 cores, fp16 on device):
  - sparse table built in c-major layout [c%128, lev, cb, t] on the DVE:
    one full-width tensor_tensor(max) per level against a flat shifted view
    (reads that run past a level slab land in never-queried entries, so no
    pads or memsets are needed).
  - each level is transposed to t-major [t%128, tt, c] on the otherwise-idle
    PE (16 identity-matmul transposes per level into fp16 PSUM), copied
    PSUM->SBUF (levels 1-3 on ACT, 4-5 on DVE), and stored to a per-batch
    DRAM row table [NLEV*T, C]. Level 0 rows come from a host-transposed
    featT input via a DRAM->DRAM copy. Batch 0's stores issue on the sync
    queue, batch 1's on the ACT queue so neither batch stalls the other.
  - the 4 RMQ terms per token (left a/b, right a/b) are fetched with 16
    hardware indirect-DMA row gathers (offsets [128, 1] int32, one 2KB row
    per partition), landing t-major in SBUF, gated on a DRAM readback
    barrier after the level stores.
  - combine on DVE: max(la, lb) + max(ra, rb); store out t-major; host
    transposes [B, T, C] -> [B, C, T].
"""

import sys
import types

import numpy as np


def _install_profile_shim():
    if "antenv.axon_hooks" in sys.modules:
        return
    try:
        hooks = types.ModuleType("antenv.axon_hooks")
        hooks._hook = None
        hooks.set_axon_ntff_profile_hook = lambda h: setattr(hooks, "_hook", h)
        hooks.get_axon_ntff_profile_hook = lambda: hooks._hook
        sys.modules["antenv.axon_hooks"] = hooks
        import antenv

        antenv.axon_hooks = hooks
        from trn_agent_boot.trn_boot import _ntff_profile_via_ctypes

        hooks.set_axon_ntff_profile_hook(
            _ntff_profile_via_ctypes("/opt/axon/libaxon_pjrt.so")
        )
    except Exception:
        pass


_install_profile_shim()

import concourse.bacc as bacc
import concourse.bass as bass
import concourse.mybir as mybir
from concourse.bass_utils import run_bass_kernel_spmd

B, C, T = 16, 1024, 256
N_CORES = 8
BPC = B // N_CORES
NLEV = 6   # sparse-table levels 0..5 (windows up to 33)
NG = 8     # indirect gathers per batch: 4 terms x 2 token chunks
CB = C // 128
LW = CB * T  # free elems per level slab per partition (2048)

_LOG2 = np.zeros(65, dtype=np.int64)
for _n in range(1, 65):
    _LOG2[_n] = _n.bit_length() - 1

_CACHE = {}
LAST_RESULT = None


def _build_graph():
    if "nc" in _CACHE:
        return _CACHE["nc"]

    nc = bacc.Bacc("TRN2", target_bir_lowering=False, debug=False,
                   num_devices=N_CORES)
    f16 = mybir.dt.float16
    i16 = mybir.dt.int16
    i32 = mybir.dt.int32

    feat_ext = nc.dram_tensor("feat16", [BPC, C, T], f16,
                              kind="ExternalInput").ap()
    featT_ext = nc.dram_tensor("featT", [BPC, T, C], f16,
                               kind="ExternalInput").ap()
    offs_ext = nc.dram_tensor("offs", [128, BPC * NG], i32,
                              kind="ExternalInput").ap()
    out_ext = nc.dram_tensor("out", [BPC, T, C], f16,
                             kind="ExternalOutput").ap()

    tbl = [nc.dram_tensor(f"tbl{b}", [NLEV * T, C], f16).ap()
           for b in range(BPC)]

    # c-major table: flat free layout [lev][cb][t]
    cbuf = [nc.alloc_sbuf_tensor(f"cbuf{b}", [128, NLEV * LW], f16).ap()
            for b in range(BPC)]
    # t-major staging, one slot per level 1..5 (slot k-1)
    tbuf = [nc.alloc_sbuf_tensor(f"tbuf{b}", [128, NLEV - 1, 2, C], f16).ap()
            for b in range(BPC)]
    gout = [nc.alloc_sbuf_tensor(f"gout{b}", [128, 4, 2, C], f16).ap()
            for b in range(BPC)]
    obuf = [nc.alloc_sbuf_tensor(f"obuf{b}", [128, 2, C], f16).ap()
            for b in range(BPC)]
    offs_sb = nc.alloc_sbuf_tensor("offs_sb", [128, BPC * NG], i32).ap()
    ident = nc.alloc_sbuf_tensor("ident", [128, 128], f16).ap()
    rb_sb = [nc.alloc_sbuf_tensor(f"rb_sb{b}", [NLEV, 64], f16).ap()
             for b in range(BPC)]

    pbuf = [[nc.alloc_psum_tensor(f"pbuf{b}_{j}", [128, 2, C], f16).ap()
             for j in range(2)] for b in range(BPC)]

    with nc.Block() as block:
        s_inc = [nc.alloc_semaphore(f"s_inc{b}") for b in range(BPC)]
        s_ino = nc.alloc_semaphore("s_ino")
        s_id = nc.alloc_semaphore("s_id")
        s_bld = [nc.alloc_semaphore(f"s_bld{b}") for b in range(BPC)]
        s_pe = [nc.alloc_semaphore(f"s_pe{b}") for b in range(BPC)]
        s_cpa = [nc.alloc_semaphore(f"s_cpa{b}") for b in range(BPC)]
        s_cpv = [nc.alloc_semaphore(f"s_cpv{b}") for b in range(BPC)]
        s_st = [nc.alloc_semaphore(f"s_st{b}") for b in range(BPC)]
        s_stq = [nc.alloc_semaphore(f"s_stq{b}") for b in range(BPC)]
        s_rb = [nc.alloc_semaphore(f"s_rb{b}") for b in range(BPC)]
        s_gl = [nc.alloc_semaphore(f"s_gl{b}") for b in range(BPC)]
        s_gr = [nc.alloc_semaphore(f"s_gr{b}") for b in range(BPC)]
        s_cmb = [nc.alloc_semaphore(f"s_cmb{b}") for b in range(BPC)]
        s_out = [nc.alloc_semaphore(f"s_out{b}") for b in range(BPC)]

        def emit_store(eng, b, k):
            if k <= 2:
                eng.wait_ge(s_cpa[b], k)
            else:
                eng.wait_ge(s_cpv[b], k - 2)
            eng.dma_start(
                out=tbl[b][k * T:(k + 1) * T].rearrange(
                    "(tt p) c -> p tt c", p=128),
                in_=tbuf[b][:, k - 1, :, :],
            ).then_inc(s_stq[b], 16)

        def emit_readback(eng, b):
            eng.wait_ge(s_st[b], 16)
            eng.wait_ge(s_stq[b], 80)
            # DRAM readback barrier: touch one chunk of every level's rows
            # before the gathers read the table
            eng.dma_start(
                out=rb_sb[b],
                in_=tbl[b].rearrange("(l t) c -> l t c", t=T)[:, 0, 0:64],
            ).then_inc(s_rb[b], 16)

        @block.sync
        def _(sync):
            for b in range(BPC):
                sync.dma_start(
                    out=cbuf[b][:, 0:LW].rearrange(
                        "p (cb t) -> p cb t", cb=CB),
                    in_=feat_ext[b].rearrange("(cb p) t -> p cb t", p=128),
                ).then_inc(s_inc[b], 16)
            sync.dma_start(out=offs_sb, in_=offs_ext).then_inc(s_ino, 16)
            for b in range(BPC):
                # level-0 rows straight from host featT (DRAM -> DRAM)
                sync.dma_start(out=tbl[b][0:T, :],
                               in_=featT_ext[b]).then_inc(s_st[b], 16)
            for k in range(1, NLEV):
                emit_store(sync, 0, k)
            emit_readback(sync, 0)
            for b in range(BPC):
                sync.wait_ge(s_cmb[b], 3)
                sync.dma_start(
                    out=out_ext[b].rearrange("(tt p) c -> p tt c", p=128),
                    in_=obuf[b][:, :, :],
                ).then_inc(s_out[b], 16)
            for b in range(BPC):
                sync.wait_ge(s_out[b], 16)

        @block.vector
        def _(vector):
            def build(b, k):
                s = 1 << (k - 1)
                if k == 1:
                    vector.wait_ge(s_inc[b], 16)
                o = (k - 1) * LW
                vector.tensor_tensor(
                    out=cbuf[b][:, k * LW:(k + 1) * LW],
                    in0=cbuf[b][:, o:o + LW],
                    in1=cbuf[b][:, o + s:o + s + LW],
                    op=mybir.AluOpType.max,
                ).then_inc(s_bld[b], 1)

            def copy_dve(b, k):
                # PSUM->SBUF copy for levels 3-5 on the DVE
                vector.wait_ge(s_pe[b], 16 * k)
                vector.tensor_copy(
                    tbuf[b][:, k - 1, :, :], pbuf[b][k % 2][:, :, :],
                ).then_inc(s_cpv[b], 1)

            for k in range(1, NLEV):
                build(0, k)
            build(1, 1)
            copy_dve(0, 3)
            build(1, 2)
            copy_dve(0, 4)
            build(1, 3)
            copy_dve(0, 5)
            build(1, 4)
            build(1, 5)
            copy_dve(1, 3)
            copy_dve(1, 4)
            copy_dve(1, 5)
            for b in range(BPC):
                # per-pair full counts (completion order across DMA engines
                # is arbitrary, so each pair gates on all 4 of its gathers)
                vector.wait_ge(s_gl[b], 64)
                vector.tensor_tensor(
                    out=gout[b][:, 0, :, :],
                    in0=gout[b][:, 0, :, :],
                    in1=gout[b][:, 1, :, :],
                    op=mybir.AluOpType.max,
                ).then_inc(s_cmb[b], 1)
                vector.wait_ge(s_gr[b], 64)
                vector.tensor_tensor(
                    out=gout[b][:, 2, :, :],
                    in0=gout[b][:, 2, :, :],
                    in1=gout[b][:, 3, :, :],
                    op=mybir.AluOpType.max,
                ).then_inc(s_cmb[b], 1)
                vector.tensor_tensor(
                    out=obuf[b][:, :, :],
                    in0=gout[b][:, 0, :, :],
                    in1=gout[b][:, 2, :, :],
                    op=mybir.AluOpType.add,
                ).then_inc(s_cmb[b], 1)

        @block.tensor
        def _(tensor):
            tensor.wait_ge(s_id, 2)
            for b in range(BPC):
                for k in range(1, NLEV):
                    tensor.wait_ge(s_bld[b], k)
                    if k in (3, 4):
                        tensor.wait_ge(s_cpa[b], k - 2)
                    elif k == 5:
                        tensor.wait_ge(s_cpv[b], 1)
                    for tt in range(2):
                        for cb in range(CB):
                            off = k * LW + cb * T + tt * 128
                            tensor.transpose(
                                out=pbuf[b][k % 2][:, tt, cb * 128:
                                                   (cb + 1) * 128],
                                in_=cbuf[b][:, off:off + 128],
                                identity=ident,
                            ).then_inc(s_pe[b], 1)

        @block.scalar
        def _(scalar):
            def copy_act(b, k):
                scalar.wait_ge(s_pe[b], 16 * k)
                scalar.copy(
                    out=tbuf[b][:, k - 1, :, :],
                    in_=pbuf[b][k % 2][:, :, :],
                ).then_inc(s_cpa[b], 1)

            copy_act(0, 1)
            copy_act(0, 2)
            copy_act(1, 1)
            emit_store(scalar, 1, 1)
            copy_act(1, 2)
            emit_store(scalar, 1, 2)
            emit_store(scalar, 1, 3)
            emit_store(scalar, 1, 4)
            emit_store(scalar, 1, 5)
            emit_readback(scalar, 1)

        @block.gpsimd
        def _(gpsimd):
            gpsimd.memset(ident, 0.0).then_inc(s_id, 1)
            gpsimd.affine_select(
                out=ident,
                in_=ident,
                compare_op=mybir.AluOpType.not_equal,
                fill=1.0,
                base=0,
                pattern=[[-1, 128]],
                channel_multiplier=1,
            ).then_inc(s_id, 1)
            gpsimd.wait_ge(s_ino, 16)
            for b in range(BPC):
                gpsimd.wait_ge(s_rb[b], 16)
                for g in range(NG):
                    sem = s_gl[b] if g < 4 else s_gr[b]
                    gpsimd.indirect_dma_start(
                        out=gout[b][:, g // 2, g % 2, :],
                        out_offset=None,
                        in_=tbl[b],
                        in_offset=bass.IndirectOffsetOnAxis(
                            ap=offs_sb[:, b * NG + g:b * NG + g + 1], axis=0),
                    ).then_inc(sem, 16)

    nc.compile()
    _CACHE["nc"] = nc
    return nc


def _host_rows(reg):
    """Table row indices [B, 4, T] for terms (la, lb, ra, rb);
    row(level, x) = level * T + x."""
    t = np.arange(T, dtype=np.int64)[None, :]

    rl = np.maximum(np.round(reg[:, :, 0]).astype(np.int64), 0)
    l_left = np.maximum(t - rl, 0)
    len_l = t + 1 - l_left
    k_l = np.where(len_l <= 64, _LOG2[np.minimum(len_l, 64)],
                   np.floor(np.log2(len_l)).astype(np.int64))
    p_l = (1 << k_l).astype(np.int64)
    la = k_l * T + l_left
    lb = k_l * T + (t + 1 - p_l)

    rr = np.clip(np.round(reg[:, :, 1]).astype(np.int64), 1, T)
    r_right = np.minimum(t + rr, T)
    len_r = r_right - t
    k_r = np.where(len_r <= 64, _LOG2[np.minimum(len_r, 64)],
                   np.floor(np.log2(len_r)).astype(np.int64))
    p_r = (1 << k_r).astype(np.int64)
    ra = k_r * T + (t + np.zeros_like(rr))
    rb = k_r * T + (r_right - p_r)

    rows = np.stack([la, lb, ra, rb], axis=1)  # [B, 4, T]
    assert rows.min() >= 0 and rows.max() < NLEV * T, (rows.min(), rows.max())
    return rows


def _wrap_idxs(flat):
    n = flat.shape[0]
    blk = flat.reshape(n // 16, 16).T
    return np.tile(blk, (8, 1))


def kernel(feat: np.ndarray, reg: np.ndarray) -> np.ndarray:
    global LAST_RESULT
    feat = np.ascontiguousarray(feat, dtype=np.float32)
    reg = np.ascontiguousarray(reg, dtype=np.float32)
    assert feat.shape == (B, C, T) and reg.shape == (B, T, 2)

    feat16 = feat.astype(np.float16)
    featT = np.ascontiguousarray(feat16.transpose(0, 2, 1))  # [B, T, C]
    rows = _host_rows(reg)  # [B, 4, T]
    # indirect offsets: offs[b][p, g], g = term*2 + tt, token t = tt*128 + p
    offs = rows.reshape(B, 4, 2, 128).reshape(B, 8, 128)
    offs = np.ascontiguousarray(offs.transpose(0, 2, 1)).astype(np.int32)

    nc = _build_graph()
    in_maps = []
    for i in range(N_CORES):
        sl = slice(i * BPC, (i + 1) * BPC)
        in_maps.append({
            "feat16": np.ascontiguousarray(feat16[sl]),
            "featT": np.ascontiguousarray(featT[sl]),
            "offs": np.ascontiguousarray(
                offs[sl].transpose(1, 0, 2).reshape(128, BPC * NG)),
        })

    res = run_bass_kernel_spmd(nc, in_maps, list(range(N_CORES)))
    LAST_RESULT = res
    outT = np.concatenate([res.results[i]["out"] for i in range(N_CORES)],
                          axis=0)  # [B, T, C] fp16
    return np.ascontiguousarray(outT.transpose(0, 2, 1)).astype(np.float32)


# revision 25
# speedup vs baseline: 1.0740x; 1.0353x over previous
"""Trainium2 Bass kernel for variable-window left/right max pooling.

out[b, c, t] = max(feat[b, c, max(t-L,0) : t+1]) + max(feat[b, c, t : min(t+R,T)])
with L = max(0, round(reg[b,t,0])), R = clip(round(reg[b,t,1]), 1, T).

Strategy (2 batches/core, pure data parallel over 8 cores, fp16 on device):
  - sparse table built in c-major layout [c%128, lev, cb, t] on the DVE:
    one full-width tensor_tensor(max) per level against a flat shifted view
    (reads that run past a level slab land in never-queried entries, so no
    pads or memsets are needed).
  - each level is transposed to t-major [t%128, tt, c] on the otherwise-idle
    PE (16 identity-matmul transposes per level into fp16 PSUM), copied
    PSUM->SBUF (levels 1-3 on ACT, 4-5 on DVE), and stored to a per-batch
    DRAM row table [NLEV*T, C]. Level 0 rows come from a host-transposed
    featT input via a DRAM->DRAM copy. Batch 0's stores issue on the sync
    queue, batch 1's on the ACT queue so neither batch stalls the other.
  - the 4 RMQ terms per token (left a/b, right a/b) are fetched with 16
    hardware indirect-DMA row gathers (offsets [128, 1] int32, one 2KB row
    per partition), landing t-major in SBUF, gated on the level stores'
    completion counts.
  - combine on DVE: max(la, lb) + max(ra, rb); store out t-major; host
    transposes [B, T, C] -> [B, C, T].
"""

import sys
import types

import numpy as np


def _install_profile_shim():
    if "antenv.axon_hooks" in sys.modules:
        return
    try:
        hooks = types.ModuleType("antenv.axon_hooks")
        hooks._hook = None
        hooks.set_axon_ntff_profile_hook = lambda h: setattr(hooks, "_hook", h)
        hooks.get_axon_ntff_profile_hook = lambda: hooks._hook
        sys.modules["antenv.axon_hooks"] = hooks
        import antenv

        antenv.axon_hooks = hooks
        from trn_agent_boot.trn_boot import _ntff_profile_via_ctypes

        hooks.set_axon_ntff_profile_hook(
            _ntff_profile_via_ctypes("/opt/axon/libaxon_pjrt.so")
        )
    except Exception:
        pass


_install_profile_shim()

import concourse.bacc as bacc
import concourse.bass as bass
import concourse.mybir as mybir
from concourse.bass_utils import run_bass_kernel_spmd

B, C, T = 16, 1024, 256
N_CORES = 8
BPC = B // N_CORES
NLEV = 6   # sparse-table levels 0..5 (windows up to 33)
NG = 8     # indirect gathers per batch: 4 terms x 2 token chunks
CB = C // 128
LW = CB * T  # free elems per level slab per partition (2048)

_LOG2 = np.zeros(65, dtype=np.int64)
for _n in range(1, 65):
    _LOG2[_n] = _n.bit_length() - 1

_CACHE = {}
LAST_RESULT = None


def _build_graph():
    if "nc" in _CACHE:
        return _CACHE["nc"]

    nc = bacc.Bacc("TRN2", target_bir_lowering=False, debug=False,
                   num_devices=N_CORES)
    f16 = mybir.dt.float16
    i16 = mybir.dt.int16
    i32 = mybir.dt.int32

    feat_ext = nc.dram_tensor("feat16", [BPC, C, T], f16,
                              kind="ExternalInput").ap()
    featT_ext = nc.dram_tensor("featT", [BPC, T, C], f16,
                               kind="ExternalInput").ap()
    offs_ext = nc.dram_tensor("offs", [128, BPC * NG], i32,
                              kind="ExternalInput").ap()
    out_ext = nc.dram_tensor("out", [BPC, T, C], f16,
                             kind="ExternalOutput").ap()

    tbl = [nc.dram_tensor(f"tbl{b}", [NLEV * T, C], f16).ap()
           for b in range(BPC)]

    # c-major table: flat free layout [lev][cb][t]
    cbuf = [nc.alloc_sbuf_tensor(f"cbuf{b}", [128, NLEV * LW], f16).ap()
            for b in range(BPC)]
    # t-major staging, one slot per level 1..5 (slot k-1)
    tbuf = [nc.alloc_sbuf_tensor(f"tbuf{b}", [128, NLEV - 1, 2, C], f16).ap()
            for b in range(BPC)]
    gout = [nc.alloc_sbuf_tensor(f"gout{b}", [128, 4, 2, C], f16).ap()
            for b in range(BPC)]
    obuf = [nc.alloc_sbuf_tensor(f"obuf{b}", [128, 2, C], f16).ap()
            for b in range(BPC)]
    offs_sb = nc.alloc_sbuf_tensor("offs_sb", [128, BPC * NG], i32).ap()
    ident = nc.alloc_sbuf_tensor("ident", [128, 128], f16).ap()

    pbuf = [[nc.alloc_psum_tensor(f"pbuf{b}_{j}", [128, 2, C], f16).ap()
             for j in range(2)] for b in range(BPC)]

    with nc.Block() as block:
        s_inc = [nc.alloc_semaphore(f"s_inc{b}") for b in range(BPC)]
        s_ino = nc.alloc_semaphore("s_ino")
        s_id = nc.alloc_semaphore("s_id")
        s_bld = [nc.alloc_semaphore(f"s_bld{b}") for b in range(BPC)]
        s_pe = [nc.alloc_semaphore(f"s_pe{b}") for b in range(BPC)]
        s_cpa = [nc.alloc_semaphore(f"s_cpa{b}") for b in range(BPC)]
        s_cpv = [nc.alloc_semaphore(f"s_cpv{b}") for b in range(BPC)]
        s_st = [nc.alloc_semaphore(f"s_st{b}") for b in range(BPC)]
        s_stq = [nc.alloc_semaphore(f"s_stq{b}") for b in range(BPC)]
        s_gl = [nc.alloc_semaphore(f"s_gl{b}") for b in range(BPC)]
        s_gr = [nc.alloc_semaphore(f"s_gr{b}") for b in range(BPC)]
        s_cmb = [nc.alloc_semaphore(f"s_cmb{b}") for b in range(BPC)]
        s_out = [nc.alloc_semaphore(f"s_out{b}") for b in range(BPC)]

        def emit_store(eng, b, k):
            if k <= 2:
                eng.wait_ge(s_cpa[b], k)
            else:
                eng.wait_ge(s_cpv[b], k - 2)
            eng.dma_start(
                out=tbl[b][k * T:(k + 1) * T].rearrange(
                    "(tt p) c -> p tt c", p=128),
                in_=tbuf[b][:, k - 1, :, :],
            ).then_inc(s_stq[b], 16)

        @block.sync
        def _(sync):
            for b in range(BPC):
                sync.dma_start(
                    out=cbuf[b][:, 0:LW].rearrange(
                        "p (cb t) -> p cb t", cb=CB),
                    in_=feat_ext[b].rearrange("(cb p) t -> p cb t", p=128),
                ).then_inc(s_inc[b], 16)
            sync.dma_start(out=offs_sb, in_=offs_ext).then_inc(s_ino, 16)
            for b in range(BPC):
                # level-0 rows straight from host featT (DRAM -> DRAM)
                sync.dma_start(out=tbl[b][0:T, :],
                               in_=featT_ext[b]).then_inc(s_st[b], 16)
            for k in range(1, NLEV):
                emit_store(sync, 0, k)
            for b in range(BPC):
                sync.wait_ge(s_cmb[b], 3)
                sync.dma_start(
                    out=out_ext[b].rearrange("(tt p) c -> p tt c", p=128),
                    in_=obuf[b][:, :, :],
                ).then_inc(s_out[b], 16)
            for b in range(BPC):
                sync.wait_ge(s_out[b], 16)

        @block.vector
        def _(vector):
            def build(b, k):
                s = 1 << (k - 1)
                if k == 1:
                    vector.wait_ge(s_inc[b], 16)
                o = (k - 1) * LW
                vector.tensor_tensor(
                    out=cbuf[b][:, k * LW:(k + 1) * LW],
                    in0=cbuf[b][:, o:o + LW],
                    in1=cbuf[b][:, o + s:o + s + LW],
                    op=mybir.AluOpType.max,
                ).then_inc(s_bld[b], 1)

            def copy_dve(b, k):
                # PSUM->SBUF copy for levels 3-5 on the DVE
                vector.wait_ge(s_pe[b], 16 * k)
                vector.tensor_copy(
                    tbuf[b][:, k - 1, :, :], pbuf[b][k % 2][:, :, :],
                ).then_inc(s_cpv[b], 1)

            for k in range(1, NLEV):
                build(0, k)
            build(1, 1)
            copy_dve(0, 3)
            build(1, 2)
            copy_dve(0, 4)
            build(1, 3)
            copy_dve(0, 5)
            build(1, 4)
            build(1, 5)
            copy_dve(1, 3)
            copy_dve(1, 4)
            copy_dve(1, 5)
            for b in range(BPC):
                # per-pair full counts (completion order across DMA engines
                # is arbitrary, so each pair gates on all 4 of its gathers)
                vector.wait_ge(s_gl[b], 64)
                vector.tensor_tensor(
                    out=gout[b][:, 0, :, :],
                    in0=gout[b][:, 0, :, :],
                    in1=gout[b][:, 1, :, :],
                    op=mybir.AluOpType.max,
                ).then_inc(s_cmb[b], 1)
                vector.wait_ge(s_gr[b], 64)
                vector.tensor_tensor(
                    out=gout[b][:, 2, :, :],
                    in0=gout[b][:, 2, :, :],
                    in1=gout[b][:, 3, :, :],
                    op=mybir.AluOpType.max,
                ).then_inc(s_cmb[b], 1)
                vector.tensor_tensor(
                    out=obuf[b][:, :, :],
                    in0=gout[b][:, 0, :, :],
                    in1=gout[b][:, 2, :, :],
                    op=mybir.AluOpType.add,
                ).then_inc(s_cmb[b], 1)

        @block.tensor
        def _(tensor):
            tensor.wait_ge(s_id, 2)
            for b in range(BPC):
                for k in range(1, NLEV):
                    tensor.wait_ge(s_bld[b], k)
                    if k in (3, 4):
                        tensor.wait_ge(s_cpa[b], k - 2)
                    elif k == 5:
                        tensor.wait_ge(s_cpv[b], 1)
                    for tt in range(2):
                        for cb in range(CB):
                            off = k * LW + cb * T + tt * 128
                            tensor.transpose(
                                out=pbuf[b][k % 2][:, tt, cb * 128:
                                                   (cb + 1) * 128],
                                in_=cbuf[b][:, off:off + 128],
                                identity=ident,
                            ).then_inc(s_pe[b], 1)

        @block.scalar
        def _(scalar):
            def copy_act(b, k):
                scalar.wait_ge(s_pe[b], 16 * k)
                scalar.copy(
                    out=tbuf[b][:, k - 1, :, :],
                    in_=pbuf[b][k % 2][:, :, :],
                ).then_inc(s_cpa[b], 1)

            copy_act(0, 1)
            copy_act(0, 2)
            copy_act(1, 1)
            emit_store(scalar, 1, 1)
            copy_act(1, 2)
            emit_store(scalar, 1, 2)
            emit_store(scalar, 1, 3)
            emit_store(scalar, 1, 4)
            emit_store(scalar, 1, 5)

        @block.gpsimd
        def _(gpsimd):
            gpsimd.memset(ident, 0.0).then_inc(s_id, 1)
            gpsimd.affine_select(
                out=ident,
                in_=ident,
                compare_op=mybir.AluOpType.not_equal,
                fill=1.0,
                base=0,
                pattern=[[-1, 128]],
                channel_multiplier=1,
            ).then_inc(s_id, 1)
            gpsimd.wait_ge(s_ino, 16)
            for b in range(BPC):
                # all 6 level stores complete (order-insensitive counts);
                # HWDGE completion semaphores imply DRAM visibility
                gpsimd.wait_ge(s_st[b], 16)
                gpsimd.wait_ge(s_stq[b], 80)
                for g in range(NG):
                    sem = s_gl[b] if g < 4 else s_gr[b]
                    gpsimd.indirect_dma_start(
                        out=gout[b][:, g // 2, g % 2, :],
                        out_offset=None,
                        in_=tbl[b],
                        in_offset=bass.IndirectOffsetOnAxis(
                            ap=offs_sb[:, b * NG + g:b * NG + g + 1], axis=0),
                    ).then_inc(sem, 16)

    nc.compile()
    _CACHE["nc"] = nc
    return nc


def _host_rows(reg):
    """Table row indices [B, 4, T] for terms (la, lb, ra, rb);
    row(level, x) = level * T + x."""
    t = np.arange(T, dtype=np.int64)[None, :]

    rl = np.maximum(np.round(reg[:, :, 0]).astype(np.int64), 0)
    l_left = np.maximum(t - rl, 0)
    len_l = t + 1 - l_left
    k_l = np.where(len_l <= 64, _LOG2[np.minimum(len_l, 64)],
                   np.floor(np.log2(len_l)).astype(np.int64))
    p_l = (1 << k_l).astype(np.int64)
    la = k_l * T + l_left
    lb = k_l * T + (t + 1 - p_l)

    rr = np.clip(np.round(reg[:, :, 1]).astype(np.int64), 1, T)
    r_right = np.minimum(t + rr, T)
    len_r = r_right - t
    k_r = np.where(len_r <= 64, _LOG2[np.minimum(len_r, 64)],
                   np.floor(np.log2(len_r)).astype(np.int64))
    p_r = (1 << k_r).astype(np.int64)
    ra = k_r * T + (t + np.zeros_like(rr))
    rb = k_r * T + (r_right - p_r)

    rows = np.stack([la, lb, ra, rb], axis=1)  # [B, 4, T]
    assert rows.min() >= 0 and rows.max() < NLEV * T, (rows.min(), rows.max())
    return rows


def _wrap_idxs(flat):
    n = flat.shape[0]
    blk = flat.reshape(n // 16, 16).T
    return np.tile(blk, (8, 1))


def kernel(feat: np.ndarray, reg: np.ndarray) -> np.ndarray:
    global LAST_RESULT
    feat = np.ascontiguousarray(feat, dtype=np.float32)
    reg = np.ascontiguousarray(reg, dtype=np.float32)
    assert feat.shape == (B, C, T) and reg.shape == (B, T, 2)

    feat16 = feat.astype(np.float16)
    featT = np.ascontiguousarray(feat16.transpose(0, 2, 1))  # [B, T, C]
    rows = _host_rows(reg)  # [B, 4, T]
    # indirect offsets: offs[b][p, g], g = term*2 + tt, token t = tt*128 + p
    offs = rows.reshape(B, 4, 2, 128).reshape(B, 8, 128)
    offs = np.ascontiguousarray(offs.transpose(0, 2, 1)).astype(np.int32)

    nc = _build_graph()
    in_maps = []
    for i in range(N_CORES):
        sl = slice(i * BPC, (i + 1) * BPC)
        in_maps.append({
            "feat16": np.ascontiguousarray(feat16[sl]),
            "featT": np.ascontiguousarray(featT[sl]),
            "offs": np.ascontiguousarray(
                offs[sl].transpose(1, 0, 2).reshape(128, BPC * NG)),
        })

    res = run_bass_kernel_spmd(nc, in_maps, list(range(N_CORES)))
    LAST_RESULT = res
    outT = np.concatenate([res.results[i]["out"] for i in range(N_CORES)],
                          axis=0)  # [B, T, C] fp16
    return np.ascontiguousarray(outT.transpose(0, 2, 1)).astype(np.float32)
